# revision 24
# baseline (speedup 1.0000x reference)
"""Trainium2 Bass kernel for MultiHeadMemAttn (mean-pooled-memory attention).

Full computation (per batch b):
    mem  = mean_pool(keyvalue, window=64, stride=64)          # [64, 512]
    hq   = query @ Wq.T ; hk = mem @ Wk.T ; hv = mem @ Wv.T   # heads=8, hd=64
    attn = softmax(hq @ hk.T / 8, over mem axis)
    out  = (attn @ hv) @ Wo.T

Sharding: data-parallel over batch across 8 cores (4 batches each),
weights replicated.  No collectives.

v2 strategy (per core):
  - ALL inputs are cast to bf16 on the host before staging: HBM traffic
    drops from 52MB to 30MB per core (the kernel is memory-regime).
    Matmul speed is identical to f32r (1 cycle/row), precision ~1e-3.
  - kv is loaded with 2 consecutive s-rows per partition so descriptors
    stay 2KB in bf16; pooling is a PE matmul against a shifted band
    matrix, accumulated over 32 half-chunks in PSUM.
  - weights are host-pre-arranged to [128, dc, 512] so each loads with
    one fully-contiguous DMA.
  - scores computed transposed (scoresT[j, i] = hkbd_p2.T @ hqT_p2), head
    pairs packed into [128, 512] tiles; softmax without max-subtraction
    (|scores| <= ~6): E = exp(s/8) on ACT (1/8 folded into host Wk),
    denominators via a ones-matmul -> [32,512], reciprocal_approx_fast on
    DVE (5x faster than reciprocal), partition-broadcast via a tiny
    expand-matmul, normalize on DVE.
  - hkbd / hvbd block-diagonal tiles are persistent ping-pong buffers;
    the zero off-diagonal blocks are written once at startup.
  - software pipeline: pool chunks of batch b+1 are woven into the
    B-phases of batch b; hq is emitted between the tail sub-steps so the
    PE never waits on ACT copies; scores of both i-chunks run before
    either B-phase so the reciprocal latency is hidden.
"""

import os
from contextlib import ExitStack

import numpy as np
import ml_dtypes

import concourse.bass as bass
import concourse.mybir as mybir
import concourse.tile as tile
from concourse.bass_utils import run_bass_kernel_spmd

F32 = mybir.dt.float32
F32R = mybir.dt.float32r
BF16 = mybir.dt.bfloat16
NPBF16 = ml_dtypes.bfloat16

NCORES = 8
B = 4          # batches per core
QLEN = 1024
S = 4096       # kv sequence length
D = 512        # hidden
H = 8          # heads
HD = 64        # head dim
MEM = 64       # mem_len (pooled length)
DC = D // 128  # 4 chunks of the hidden dim
ICN = 2        # i-chunks of 512 per batch
IT = 4         # 128-row tiles per i-chunk
KT = 8         # kv DMA tiles per batch (each 512 s-rows: 2 groups x 128p x 2rows)

EXPF = mybir.ActivationFunctionType.Exp

# ---------------------------------------------------------------------------
# Workaround: this walrus build only encodes ONE sem-wait per instruction
# ("Too many sync wait commands" in CoreV3GenImpl setupSyncWait), while
# Tile's sem-assignment freely attaches several.  Post-process the
# serialized BIR: move surplus waits onto injected same-engine NoOps placed
# immediately before the instruction (engine streams are in-order, so the
# NoOp chain stalls the engine exactly like multi-wait would).
import json as _json

_orig_to_json_bytes = bass.Bass.to_json_bytes


def _split_multi_waits(self, *args, **kwargs):
    raw = _orig_to_json_bytes(self, *args, **kwargs)
    d = _json.loads(raw)
    changed = False

    def fix_block(o):
        nonlocal changed
        if isinstance(o, dict):
            insts = o.get("instructions")
            if isinstance(insts, list):
                new = []
                for inst in insts:
                    si = inst.get("sync_info") if isinstance(inst, dict) else None
                    waits = (si or {}).get("on_wait") or []
                    if len(waits) > 1:
                        changed = True
                        for i, w in enumerate(waits[:-1]):
                            new.append(
                                {
                                    "name": f"{inst['name']}-sw{i}",
                                    "opcode": "NoOp",
                                    "engine": inst["engine"],
                                    "ins": [],
                                    "outs": [],
                                    "debug": inst.get("debug", 0),
                                    "sync_info": {
                                        "on_wait": [w],
                                        "on_update": [],
                                    },
                                }
                            )
                        si["on_wait"] = [waits[-1]]
                    new.append(inst)
                o["instructions"] = new
            for v in o.values():
                fix_block(v)
        elif isinstance(o, list):
            for v in o:
                fix_block(v)

    fix_block(d)
    if not changed:
        return raw
    return _json.dumps(d).encode()


bass.Bass.to_json_bytes = _split_multi_waits
# ---------------------------------------------------------------------------


def _build_nc() -> bass.Bass:
    nc = bass.Bass()
    q = nc.dram_tensor("queryT", [B, D, QLEN], BF16, kind="ExternalInput")
    kv = nc.dram_tensor("keyvalue", [B, S, D], BF16, kind="ExternalInput")
    wq = nc.dram_tensor("wq", [128, DC, D], BF16, kind="ExternalInput")
    wk = nc.dram_tensor("wk", [128, DC, D], BF16, kind="ExternalInput")
    wv = nc.dram_tensor("wv", [128, DC, D], BF16, kind="ExternalInput")
    wo = nc.dram_tensor("wo", [128, DC, D], BF16, kind="ExternalInput")
    poolD = nc.dram_tensor("poolD", [128, 124], BF16, kind="ExternalInput")
    ident = nc.dram_tensor("ident", [MEM, MEM], BF16, kind="ExternalInput")
    ones2 = nc.dram_tensor("ones2", [128, 4, 32], BF16, kind="ExternalInput")
    expand2 = nc.dram_tensor("expand2", [32, 4, 128], F32R, kind="ExternalInput")
    out = nc.dram_tensor("out", [B, QLEN, D], F32, kind="ExternalOutput")

    # DRAM views for partition-major DMA (2KB descriptors everywhere)
    q_v = q.rearrange("b (dc p) i -> b p dc i", p=128)
    kv_v = kv.rearrange("b (t c p two) d -> b t p c two d", t=KT, c=2, p=128, two=2)
    out_v = out.rearrange("b (ic it p) d -> b ic p it d", ic=ICN, it=IT, p=128)

    with tile.TileContext(nc) as tc, ExitStack() as ctx:
        # SBUF pools
        singles = ctx.enter_context(tc.tile_pool(name="singles", bufs=1))
        kvp = ctx.enter_context(tc.tile_pool(name="kvp", bufs=8))
        qtp = ctx.enter_context(tc.tile_pool(name="qtp", bufs=2))
        hqp = ctx.enter_context(tc.tile_pool(name="hqp", bufs=2))
        memp = ctx.enter_context(tc.tile_pool(name="memp", bufs=2))
        ep = ctx.enter_context(tc.tile_pool(name="ep", bufs=9))
        attnp = ctx.enter_context(tc.tile_pool(name="attnp", bufs=4))
        rdp = ctx.enter_context(tc.tile_pool(name="rdp", bufs=3))
        vtp = ctx.enter_context(tc.tile_pool(name="vtp", bufs=3))
        outp = ctx.enter_context(tc.tile_pool(name="outp", bufs=2))
        # PSUM pools (8 banks total: 1 + 1 + 6)
        accp = ctx.enter_context(tc.tile_pool(name="accp", bufs=1, space="PSUM"))
        denp = ctx.enter_context(tc.tile_pool(name="denp", bufs=1, space="PSUM"))
        mmp = ctx.enter_context(tc.tile_pool(name="mmp", bufs=6, space="PSUM"))

        # --- one-time loads; small consts go on the scalar (ACT) HWDGE ring
        # so the first kv tile starts on the sync ring immediately ---
        poolD_sb = singles.tile([128, 124], BF16)
        nc.scalar.dma_start(out=poolD_sb, in_=poolD[:, :])
        ident_sb = singles.tile([MEM, MEM], BF16)
        nc.scalar.dma_start(out=ident_sb, in_=ident[:, :])

        # persistent block-diagonal tiles (ping-pong); zero the whole tiles
        # once, per-batch writes touch only the diagonal blocks.
        hkbd_t = [singles.tile([128, 4, 128], BF16, name=f"hkbd{i}") for i in range(2)]
        hvbd_t = [singles.tile([128, 4, 128], BF16, name=f"hvbd{i}") for i in range(2)]
        for t in hkbd_t + hvbd_t:
            nc.gpsimd.memset(t[:, :, :], 0.0)

        wq_sb = singles.tile([128, DC, D], BF16)
        wk_sb = singles.tile([128, DC, D], BF16)
        wv_sb = singles.tile([128, DC, D], BF16)
        wo_sb = singles.tile([128, DC, D], BF16)
        ones2_sb = singles.tile([128, 4, 32], BF16)
        expand2_sb = singles.tile([32, 4, 128], F32R)

        def make_prep(b):
            """Returns (chunk_steps, state).  Each chunk step loads one kv
            tile (512 s-rows as [128p, 2c, 2rows, 512d]) and runs its 4 pool
            matmuls; steps are interleaved into other units' emission."""
            st = {"b": b, "bd": b % 2}
            pacc = accp.tile([MEM, D], F32, tag="acc")
            st["pacc"] = pacc

            def chunk_step(t):
                def f():
                    kvt = kvp.tile([128, 2, 2, D], BF16, tag="kv")
                    nc.sync.dma_start(out=kvt, in_=kv_v[b, t])
                    for c in range(2):
                        g = 2 * t + c
                        for two in range(2):
                            nc.tensor.matmul(
                                pacc,
                                lhsT=poolD_sb[:, 60 - 4 * g : 124 - 4 * g],
                                rhs=kvt[:, c, two, :],
                                start=(g == 0 and two == 0),
                                stop=(g == 15 and two == 1),
                            )
                return f

            return [chunk_step(t) for t in range(KT)], st

        def tail_a(bst):
            """mem copy + PE transposes + memT copy."""
            mem_sb = memp.tile([MEM, D], BF16, tag="mem")
            nc.scalar.copy(out=mem_sb, in_=bst["pacc"])
            trt = mmp.tile([128, 4, MEM], BF16, tag="mm")
            for c in range(4):
                nc.tensor.transpose(
                    trt[:, c, :],
                    mem_sb[:, 128 * c : 128 * (c + 1)],
                    ident_sb[:, :],
                )
            memT_sb = memp.tile([128, 4, MEM], BF16, tag="memT")
            nc.scalar.copy(out=memT_sb, in_=trt)
            bst["memT"] = memT_sb

        def tail_b(bst):
            """hk -> hkbd diag, hv -> hvbd diag (ping-pong tiles)."""
            memT_sb = bst["memT"]
            hkbd_sb = hkbd_t[bst["bd"]]
            hvbd_sb = hvbd_t[bst["bd"]]
            for oc in range(4):
                hk_ps = mmp.tile([128, MEM], F32, tag="mm")
                for dc in range(DC):
                    nc.tensor.matmul(
                        hk_ps,
                        lhsT=wk_sb[:, dc, 128 * oc : 128 * (oc + 1)],
                        rhs=memT_sb[:, dc, :],
                        start=(dc == 0),
                        stop=(dc == DC - 1),
                    )
                # 1/8 score scale is folded into host wk
                nc.scalar.copy(out=hkbd_sb[0:64, oc, 0:64], in_=hk_ps[0:64, :])
                nc.scalar.copy(out=hkbd_sb[64:128, oc, 64:128], in_=hk_ps[64:128, :])
            hv_ps = mmp.tile([MEM, D], F32, tag="mm")
            for dc in range(DC):
                nc.tensor.matmul(
                    hv_ps,
                    lhsT=memT_sb[:, dc, :],
                    rhs=wv_sb[:, dc, :],
                    start=(dc == 0),
                    stop=(dc == DC - 1),
                )
            hv_sb = memp.tile([MEM, D], BF16, tag="hv")
            nc.scalar.copy(out=hv_sb, in_=hv_ps)
            ev = hv_sb.rearrange("m (p2 two dv) -> m p2 two dv", p2=4, two=2)
            nc.scalar.copy(out=hvbd_sb[0:64, :, 0:64], in_=ev[:, :, 0, :])
            nc.sync.dma_start(out=hvbd_sb[64:128, :, 64:128], in_=ev[:, :, 1, :])
            bst["hkbd"] = hkbd_sb
            bst["hvbd"] = hvbd_sb

        def emit_hq(bst, qT_sb, ic):
            """hqT[:, oc, :] = Wq-chunks.T @ qT for one 512-wide i-chunk."""
            hqT_sb = hqp.tile([128, DC, D], BF16, tag="hqT")
            for oc in range(DC):
                hq_ps = mmp.tile([128, D], F32, tag="mm")
                for dc in range(DC):
                    nc.tensor.matmul(
                        hq_ps,
                        lhsT=wq_sb[:, dc, 128 * oc : 128 * (oc + 1)],
                        rhs=qT_sb[:, dc, 512 * ic : 512 * (ic + 1)],
                        start=(dc == 0),
                        stop=(dc == DC - 1),
                    )
                # split ACT/DVE so neither queue stacks up before the exps
                if oc % 2 == 0:
                    nc.vector.tensor_copy(out=hqT_sb[:, oc, :], in_=hq_ps)
                else:
                    nc.scalar.copy(out=hqT_sb[:, oc, :], in_=hq_ps)
            return hqT_sb

        def emit_score(bst, hqT_sb, b, ic, filler=None):
            """scores, exp, denominators, reciprocal for one unit."""
            st = {"b": b, "ic": ic, "bst": bst}
            den_ps = denp.tile([32, D], F32, tag="den")
            e_tiles = []
            # two passes: all scores+exp first, then all den matmuls, so the
            # in-order PE queue never parks on a den waiting for its exp; a
            # pool chunk of the next batch between them paces PE to ACT.
            for p2 in range(4):
                sc_ps = mmp.tile([128, D], F32, tag="mm")
                nc.tensor.matmul(
                    sc_ps,
                    lhsT=bst["hkbd"][:, p2, :],
                    rhs=hqT_sb[:, p2, :],
                    start=True,
                    stop=True,
                )
                e_sb = ep.tile([128, D], BF16, tag="e")
                nc.scalar.activation(out=e_sb, in_=sc_ps, func=EXPF)
                e_tiles.append(e_sb)
            if filler:
                filler.pop(0)()
            for p2 in range(4):
                nc.tensor.matmul(
                    den_ps,
                    lhsT=ones2_sb[:, p2, :],
                    rhs=e_tiles[p2],
                    start=(p2 == 0),
                    stop=(p2 == 3),
                )
            # 1/den = exp(-ln(den)) on ACT: keeps the DVE queue free for the
            # attn multiplies and is ~3x cheaper than DVE reciprocal.
            lnd = rdp.tile([32, D], F32, tag="lnd")
            nc.scalar.activation(out=lnd, in_=den_ps, func=mybir.ActivationFunctionType.Ln)
            rden = rdp.tile([32, D], F32R, tag="rden")
            nc.scalar.activation(out=rden, in_=lnd, func=EXPF, scale=-1.0)
            st["e"] = e_tiles
            st["rden"] = rden
            return st

        def emit_B(st, filler, split_store=False):
            """normalize, V-matmul, Wo projection, store for unit st.

            The out-projection accumulates per head-chunk (start/stop over
            hc) so its matmuls interleave with the bc/mul/uv chain: the PE
            always has out-work while DVE runs the attn multiplies.  The
            DVE does ONLY the 4 multiplies; all copies go to ACT so the
            mul chain is never delayed."""
            b, ic, bst = st["b"], st["ic"], st["bst"]
            vecT_sb = vtp.tile([128, 4, D], BF16, tag="vecT")

            def bc_mul(p2):
                bc_ps = mmp.tile([128, D], F32, tag="mm")
                nc.tensor.matmul(
                    bc_ps,
                    lhsT=expand2_sb[:, p2, :],
                    rhs=st["rden"],
                    start=True,
                    stop=True,
                )
                attn_sb = attnp.tile([128, D], BF16, tag="attn")
                nc.vector.tensor_mul(attn_sb, st["e"][p2], bc_ps)
                return attn_sb

            def uv(p2, attn_sb):
                uv_ps = mmp.tile([128, D], F32, tag="mm")
                nc.tensor.matmul(
                    uv_ps,
                    lhsT=bst["hvbd"][:, p2, :],
                    rhs=attn_sb,
                    start=True,
                    stop=True,
                )
                nc.scalar.copy(out=vecT_sb[:, p2, :], in_=uv_ps)

            o_ps = {}

            def out_hc(hc):
                for it in range(IT):
                    if hc == 0:
                        o_ps[it] = mmp.tile([128, D], F32, tag="mm", name=f"ops{it}")
                    nc.tensor.matmul(
                        o_ps[it],
                        lhsT=vecT_sb[:, hc, 128 * it : 128 * (it + 1)],
                        rhs=wo_sb[:, hc, :],
                        start=(hc == 0),
                        stop=(hc == 3),
                    )

            a0 = bc_mul(0)
            a1 = bc_mul(1)
            if filler:
                filler.pop(0)()
            uv(0, a0)
            a2 = bc_mul(2)
            uv(1, a1)
            a3 = bc_mul(3)
            out_hc(0)
            uv(2, a2)
            out_hc(1)
            uv(3, a3)
            out_hc(2)
            if filler:
                filler.pop(0)()
            out_hc(3)
            out_sb = outp.tile([128, IT, D], F32, tag="o")
            for it in range(IT):
                if it % 2 == 0:
                    nc.vector.tensor_copy(out=out_sb[:, it, :], in_=o_ps[it])
                else:
                    nc.scalar.copy(out=out_sb[:, it, :], in_=o_ps[it])
                if split_store:
                    nc.scalar.dma_start(
                        out=out_v[b, ic][:, it, :], in_=out_sb[:, it, :]
                    )
            if not split_store:
                # store on the second HWDGE ring (ACT) so loads/stores overlap
                nc.scalar.dma_start(out=out_v[b, ic], in_=out_sb)

        # ------------------- software pipeline -------------------
        # batch 0: kv chunks + consts first, hq woven between tail steps
        steps0, bst0 = make_prep(0)
        steps0.pop(0)()
        steps0.pop(0)()
        qT0 = qtp.tile([128, DC, QLEN], BF16, tag="qT", name="qT0")
        nc.sync.dma_start(out=qT0, in_=q_v[0])
        steps0.pop(0)()
        steps0.pop(0)()
        nc.sync.dma_start(out=wq_sb, in_=wq[:, :, :])
        hqT_00 = emit_hq(bst0, qT0, 0)
        steps0.pop(0)()
        steps0.pop(0)()
        nc.sync.dma_start(out=wk_sb, in_=wk[:, :, :])
        nc.sync.dma_start(out=wv_sb, in_=wv[:, :, :])
        steps0.pop(0)()
        steps0.pop(0)()
        nc.scalar.dma_start(out=ones2_sb, in_=ones2[:, :, :])
        nc.scalar.dma_start(out=expand2_sb, in_=expand2[:, :, :])
        tail_a(bst0)
        hqT_01 = emit_hq(bst0, qT0, 1)
        nc.sync.dma_start(out=wo_sb, in_=wo[:, :, :])
        tail_b(bst0)

        batch_state = {0: bst0}
        hq_pre = {(0, 0): hqT_00, (0, 1): hqT_01}
        qT_cur = qT0
        for b in range(B):
            bst = batch_state[b]
            if b > 0:
                hq_pre[(b, 0)] = emit_hq(bst, qT_cur, 0)
                tail_a(bst)
                if b < B - 1:
                    # last batch defers hq(b,1) to fill the score phase
                    # (it has no next-batch pool chunks to weave)
                    hq_pre[(b, 1)] = emit_hq(bst, qT_cur, 1)
                tail_b(bst)
            if b + 1 < B:
                next_steps, next_bst = make_prep(b + 1)
                qT_next = qtp.tile([128, DC, QLEN], BF16, tag="qT")
                nc.sync.dma_start(out=qT_next, in_=q_v[b + 1])
            else:
                next_steps, next_bst, qT_next = [], None, None
            st0 = emit_score(bst, hq_pre.pop((b, 0)), b, 0, next_steps)
            if (b, 1) not in hq_pre:
                hq_pre[(b, 1)] = emit_hq(bst, qT_cur, 1)
            st1 = emit_score(bst, hq_pre.pop((b, 1)), b, 1, next_steps)
            # two more chunks here cover the den->rden ACT latency before
            # B(b,0)'s first broadcast matmul needs rden.
            for _ in range(min(2, len(next_steps))):
                next_steps.pop(0)()
            emit_B(st0, next_steps)
            emit_B(st1, next_steps, split_store=(b == B - 1))
            if next_bst is not None:
                batch_state[b + 1] = next_bst
                qT_cur = qT_next
    return nc


_NC = None


def _get_nc() -> bass.Bass:
    global _NC
    if _NC is None:
        _NC = _build_nc()
    return _NC


def _consts() -> dict:
    # pooling band: slice [:, 60-4g : 124-4g] has 1/64 at column (4g + p//32)
    poolD = np.zeros((128, 124), np.float32)
    for p in range(128):
        poolD[p, 60 + p // 32] = 1.0 / 64.0
    ident = np.eye(MEM, dtype=np.float32)
    # den matmul lhsT (per pair p2): row 2*p2 sums even-head exp rows
    # (partitions 0-63), row 2*p2+1 sums odd (64-127); rows 8-31 duplicate
    # row 0's pattern so every PSUM row stays finite positive (the
    # approx-reciprocal runs on the whole tile and inf/0 are undefined).
    ones2 = np.zeros((128, 4, 32), np.float32)
    for p2 in range(4):
        ones2[0:64, p2, 2 * p2] = 1.0
        ones2[64:128, p2, 2 * p2 + 1] = 1.0
        if p2 == 0:
            ones2[0:64, p2, 8:32] = 1.0
    # broadcast matmul lhsT, one [32, 128] slice per head pair
    expand2 = np.zeros((32, 4, 128), np.float32)
    for p2 in range(4):
        expand2[2 * p2 + 0, p2, 0:64] = 1.0
        expand2[2 * p2 + 1, p2, 64:128] = 1.0
    return {
        "poolD": poolD.astype(NPBF16),
        "ident": ident.astype(NPBF16),
        "ones2": ones2.astype(NPBF16),
        "expand2": expand2,
    }


def _w_layout(wT: np.ndarray, scale: float = 1.0) -> np.ndarray:
    """[D, D] (already transposed) -> [128, DC, D] bf16, contiguous."""
    w = (wT * scale).astype(NPBF16) if scale != 1.0 else wT.astype(NPBF16)
    return np.ascontiguousarray(w.reshape(DC, 128, D).transpose(1, 0, 2))


def run(inputs: dict, trace: bool = False):
    """Run on 8 cores; returns (full_output, BassKernelResults)."""
    query = np.asarray(inputs["query"], np.float32)
    queryT = np.ascontiguousarray(query.transpose(0, 2, 1)).astype(NPBF16)
    keyvalue = np.asarray(inputs["keyvalue"], np.float32).astype(NPBF16)
    w = {
        "wq": _w_layout(np.asarray(inputs["Wq"], np.float32).T),
        # fold the 1/sqrt(hd)=1/8 score scale into Wk (exact in bf16)
        "wk": _w_layout(np.asarray(inputs["Wk"], np.float32).T, 0.125),
        "wv": _w_layout(np.asarray(inputs["Wv"], np.float32).T),
        "wo": _w_layout(np.asarray(inputs["Wo"], np.float32).T),
    }
    consts = _consts()
    nb = query.shape[0]
    per = nb // NCORES
    assert per == B, f"expected {NCORES * B} batches, got {nb}"

    in_maps = []
    for k in range(NCORES):
        m = {
            "queryT": np.ascontiguousarray(queryT[k * per : (k + 1) * per]),
            "keyvalue": np.ascontiguousarray(keyvalue[k * per : (k + 1) * per]),
        }
        m.update(w)
        m.update(consts)
        in_maps.append(m)

    res = run_bass_kernel_spmd(
        _get_nc(), in_maps, core_ids=list(range(NCORES)), trace=trace
    )
    outs = [r["out"] for r in res.results]
    return np.concatenate(outs, axis=0), res


def kernel(**inputs) -> np.ndarray:
    out, _ = run(inputs, trace=False)
    return out


# revision 27
# speedup vs baseline: 1.0213x; 1.0213x over previous
"""Trainium2 Bass kernel for MultiHeadMemAttn (mean-pooled-memory attention).

Full computation (per batch b):
    mem  = mean_pool(keyvalue, window=64, stride=64)          # [64, 512]
    hq   = query @ Wq.T ; hk = mem @ Wk.T ; hv = mem @ Wv.T   # heads=8, hd=64
    attn = softmax(hq @ hk.T / 8, over mem axis)
    out  = (attn @ hv) @ Wo.T

Sharding: data-parallel over batch across 8 cores (4 batches each),
weights replicated.  No collectives.

v2 strategy (per core):
  - ALL inputs are cast to bf16 on the host before staging: HBM traffic
    drops from 52MB to 30MB per core (the kernel is memory-regime).
    Matmul speed is identical to f32r (1 cycle/row), precision ~1e-3.
  - kv is loaded with 2 consecutive s-rows per partition so descriptors
    stay 2KB in bf16; pooling is a PE matmul against a shifted band
    matrix, accumulated over 32 half-chunks in PSUM.
  - weights are host-pre-arranged to [128, dc, 512] so each loads with
    one fully-contiguous DMA.
  - scores computed transposed (scoresT[j, i] = hkbd_p2.T @ hqT_p2), head
    pairs packed into [128, 512] tiles; softmax without max-subtraction
    (|scores| <= ~6): E = exp(s/8) on ACT (1/8 folded into host Wk),
    denominators via a ones-matmul -> [32,512], reciprocal_approx_fast on
    DVE (5x faster than reciprocal), partition-broadcast via a tiny
    expand-matmul, normalize on DVE.
  - hkbd / hvbd block-diagonal tiles are persistent ping-pong buffers;
    the zero off-diagonal blocks are written once at startup.
  - software pipeline: pool chunks of batch b+1 are woven into the
    B-phases of batch b; hq is emitted between the tail sub-steps so the
    PE never waits on ACT copies; scores of both i-chunks run before
    either B-phase so the reciprocal latency is hidden.
"""

import os
from contextlib import ExitStack

import numpy as np
import ml_dtypes

import concourse.bass as bass
import concourse.mybir as mybir
import concourse.tile as tile
from concourse.bass_utils import run_bass_kernel_spmd

F32 = mybir.dt.float32
F32R = mybir.dt.float32r
BF16 = mybir.dt.bfloat16
NPBF16 = ml_dtypes.bfloat16

NCORES = 8
B = 4          # batches per core
QLEN = 1024
S = 4096       # kv sequence length
D = 512        # hidden
H = 8          # heads
HD = 64        # head dim
MEM = 64       # mem_len (pooled length)
DC = D // 128  # 4 chunks of the hidden dim
ICN = 2        # i-chunks of 512 per batch
IT = 4         # 128-row tiles per i-chunk
KT = 8         # kv DMA tiles per batch (each 512 s-rows: 2 groups x 128p x 2rows)

EXPF = mybir.ActivationFunctionType.Exp

# ---------------------------------------------------------------------------
# Workaround: this walrus build only encodes ONE sem-wait per instruction
# ("Too many sync wait commands" in CoreV3GenImpl setupSyncWait), while
# Tile's sem-assignment freely attaches several.  Post-process the
# serialized BIR: move surplus waits onto injected same-engine NoOps placed
# immediately before the instruction (engine streams are in-order, so the
# NoOp chain stalls the engine exactly like multi-wait would).
import json as _json

_orig_to_json_bytes = bass.Bass.to_json_bytes


def _split_multi_waits(self, *args, **kwargs):
    raw = _orig_to_json_bytes(self, *args, **kwargs)
    d = _json.loads(raw)
    changed = False

    def fix_block(o):
        nonlocal changed
        if isinstance(o, dict):
            insts = o.get("instructions")
            if isinstance(insts, list):
                new = []
                for inst in insts:
                    si = inst.get("sync_info") if isinstance(inst, dict) else None
                    waits = (si or {}).get("on_wait") or []
                    if len(waits) > 1:
                        changed = True
                        for i, w in enumerate(waits[:-1]):
                            new.append(
                                {
                                    "name": f"{inst['name']}-sw{i}",
                                    "opcode": "NoOp",
                                    "engine": inst["engine"],
                                    "ins": [],
                                    "outs": [],
                                    "debug": inst.get("debug", 0),
                                    "sync_info": {
                                        "on_wait": [w],
                                        "on_update": [],
                                    },
                                }
                            )
                        si["on_wait"] = [waits[-1]]
                    new.append(inst)
                o["instructions"] = new
            for v in o.values():
                fix_block(v)
        elif isinstance(o, list):
            for v in o:
                fix_block(v)

    fix_block(d)
    if not changed:
        return raw
    return _json.dumps(d).encode()


bass.Bass.to_json_bytes = _split_multi_waits
# ---------------------------------------------------------------------------


def _build_nc() -> bass.Bass:
    nc = bass.Bass()
    q = nc.dram_tensor("queryT", [B, D, QLEN], BF16, kind="ExternalInput")
    kv = nc.dram_tensor("keyvalue", [B, S, D], BF16, kind="ExternalInput")
    wq = nc.dram_tensor("wq", [128, DC, D], BF16, kind="ExternalInput")
    wk = nc.dram_tensor("wk", [128, DC, D], BF16, kind="ExternalInput")
    wv = nc.dram_tensor("wv", [128, DC, D], BF16, kind="ExternalInput")
    wo = nc.dram_tensor("wo", [128, DC, D], BF16, kind="ExternalInput")
    poolD = nc.dram_tensor("poolD", [128, 124], BF16, kind="ExternalInput")
    ident = nc.dram_tensor("ident", [MEM, MEM], BF16, kind="ExternalInput")
    ones2 = nc.dram_tensor("ones2", [128, 4, 32], BF16, kind="ExternalInput")
    expand2 = nc.dram_tensor("expand2", [32, 4, 128], F32R, kind="ExternalInput")
    out = nc.dram_tensor("out", [B, QLEN, D], F32, kind="ExternalOutput")

    # DRAM views for partition-major DMA (2KB descriptors everywhere)
    q_v = q.rearrange("b (dc p) i -> b p dc i", p=128)
    kv_v = kv.rearrange("b (t c p two) d -> b t p c two d", t=KT, c=2, p=128, two=2)
    out_v = out.rearrange("b (ic it p) d -> b ic p it d", ic=ICN, it=IT, p=128)

    with tile.TileContext(nc) as tc, ExitStack() as ctx:
        # SBUF pools
        singles = ctx.enter_context(tc.tile_pool(name="singles", bufs=1))
        kvp = ctx.enter_context(tc.tile_pool(name="kvp", bufs=8))
        qtp = ctx.enter_context(tc.tile_pool(name="qtp", bufs=2))
        hqp = ctx.enter_context(tc.tile_pool(name="hqp", bufs=2))
        memp = ctx.enter_context(tc.tile_pool(name="memp", bufs=2))
        ep = ctx.enter_context(tc.tile_pool(name="ep", bufs=9))
        attnp = ctx.enter_context(tc.tile_pool(name="attnp", bufs=4))
        rdp = ctx.enter_context(tc.tile_pool(name="rdp", bufs=3))
        vtp = ctx.enter_context(tc.tile_pool(name="vtp", bufs=3))
        outp = ctx.enter_context(tc.tile_pool(name="outp", bufs=2))
        # PSUM pools (8 banks total: 1 + 1 + 6)
        accp = ctx.enter_context(tc.tile_pool(name="accp", bufs=1, space="PSUM"))
        denp = ctx.enter_context(tc.tile_pool(name="denp", bufs=1, space="PSUM"))
        mmp = ctx.enter_context(tc.tile_pool(name="mmp", bufs=6, space="PSUM"))

        # --- one-time loads; small consts go on the scalar (ACT) HWDGE ring
        # so the first kv tile starts on the sync ring immediately ---
        poolD_sb = singles.tile([128, 124], BF16)
        nc.scalar.dma_start(out=poolD_sb, in_=poolD[:, :])
        ident_sb = singles.tile([MEM, MEM], BF16)
        nc.scalar.dma_start(out=ident_sb, in_=ident[:, :])

        # persistent block-diagonal tiles (ping-pong); zero the whole tiles
        # once, per-batch writes touch only the diagonal blocks.
        hkbd_t = [singles.tile([128, 4, 128], BF16, name=f"hkbd{i}") for i in range(2)]
        hvbd_t = [singles.tile([128, 4, 128], BF16, name=f"hvbd{i}") for i in range(2)]
        for t in hkbd_t + hvbd_t:
            nc.gpsimd.memset(t[:, :, :], 0.0)

        wq_sb = singles.tile([128, DC, D], BF16)
        wk_sb = singles.tile([128, DC, D], BF16)
        wv_sb = singles.tile([128, DC, D], BF16)
        wo_sb = singles.tile([128, DC, D], BF16)
        ones2_sb = singles.tile([128, 4, 32], BF16)
        expand2_sb = singles.tile([32, 4, 128], F32R)

        def make_prep(b, halves=False):
            """Returns (chunk_steps, state).  Each chunk step loads one kv
            tile (512 s-rows as [128p, 2c, 2rows, 512d]) and runs its 4 pool
            matmuls; steps are interleaved into other units' emission.
            halves=True (batch 0) uses 16 quarter-MB steps so the first
            pool matmul starts as early as possible."""
            st = {"b": b, "bd": b % 2}
            pacc = accp.tile([MEM, D], F32, tag="acc")
            st["pacc"] = pacc

            def chunk_step(t):
                def f():
                    kvt = kvp.tile([128, 2, 2, D], BF16, tag="kv")
                    nc.sync.dma_start(out=kvt, in_=kv_v[b, t])
                    for c in range(2):
                        g = 2 * t + c
                        for two in range(2):
                            nc.tensor.matmul(
                                pacc,
                                lhsT=poolD_sb[:, 60 - 4 * g : 124 - 4 * g],
                                rhs=kvt[:, c, two, :],
                                start=(g == 0 and two == 0),
                                stop=(g == 15 and two == 1),
                            )
                return f

            def half_step(t, c):
                def f():
                    kvh = kvp.tile([128, 2, D], BF16, tag="kvh")
                    nc.sync.dma_start(out=kvh, in_=kv_v[b, t][:, c, :, :])
                    g = 2 * t + c
                    for two in range(2):
                        nc.tensor.matmul(
                            pacc,
                            lhsT=poolD_sb[:, 60 - 4 * g : 124 - 4 * g],
                            rhs=kvh[:, two, :],
                            start=(g == 0 and two == 0),
                            stop=(g == 15 and two == 1),
                        )
                return f

            if halves:
                return [half_step(t, c) for t in range(KT) for c in range(2)], st
            return [chunk_step(t) for t in range(KT)], st

        def tail_a(bst):
            """mem copy + PE transposes + memT copy."""
            mem_sb = memp.tile([MEM, D], BF16, tag="mem")
            nc.scalar.copy(out=mem_sb, in_=bst["pacc"])
            trt = mmp.tile([128, 4, MEM], BF16, tag="mm")
            for c in range(4):
                nc.tensor.transpose(
                    trt[:, c, :],
                    mem_sb[:, 128 * c : 128 * (c + 1)],
                    ident_sb[:, :],
                )
            memT_sb = memp.tile([128, 4, MEM], BF16, tag="memT")
            nc.scalar.copy(out=memT_sb, in_=trt)
            bst["memT"] = memT_sb

        def tail_b(bst):
            """hk -> hkbd diag, hv -> hvbd diag (ping-pong tiles)."""
            memT_sb = bst["memT"]
            hkbd_sb = hkbd_t[bst["bd"]]
            hvbd_sb = hvbd_t[bst["bd"]]
            for oc in range(4):
                hk_ps = mmp.tile([128, MEM], F32, tag="mm")
                for dc in range(DC):
                    nc.tensor.matmul(
                        hk_ps,
                        lhsT=wk_sb[:, dc, 128 * oc : 128 * (oc + 1)],
                        rhs=memT_sb[:, dc, :],
                        start=(dc == 0),
                        stop=(dc == DC - 1),
                    )
                # 1/8 score scale is folded into host wk
                nc.scalar.copy(out=hkbd_sb[0:64, oc, 0:64], in_=hk_ps[0:64, :])
                nc.scalar.copy(out=hkbd_sb[64:128, oc, 64:128], in_=hk_ps[64:128, :])
            hv_ps = mmp.tile([MEM, D], F32, tag="mm")
            for dc in range(DC):
                nc.tensor.matmul(
                    hv_ps,
                    lhsT=memT_sb[:, dc, :],
                    rhs=wv_sb[:, dc, :],
                    start=(dc == 0),
                    stop=(dc == DC - 1),
                )
            hv_sb = memp.tile([MEM, D], BF16, tag="hv")
            nc.scalar.copy(out=hv_sb, in_=hv_ps)
            ev = hv_sb.rearrange("m (p2 two dv) -> m p2 two dv", p2=4, two=2)
            nc.scalar.copy(out=hvbd_sb[0:64, :, 0:64], in_=ev[:, :, 0, :])
            nc.sync.dma_start(out=hvbd_sb[64:128, :, 64:128], in_=ev[:, :, 1, :])
            bst["hkbd"] = hkbd_sb
            bst["hvbd"] = hvbd_sb

        def emit_hq(bst, qT_sb, ic, ocs=None, hqT_sb=None):
            """hqT[:, oc, :] = Wq-chunks.T @ qT for one 512-wide i-chunk."""
            if hqT_sb is None:
                hqT_sb = hqp.tile([128, DC, D], BF16, tag="hqT", name="hqT_t")
            for oc in ocs if ocs is not None else range(DC):
                hq_ps = mmp.tile([128, D], F32, tag="mm")
                for dc in range(DC):
                    nc.tensor.matmul(
                        hq_ps,
                        lhsT=wq_sb[:, dc, 128 * oc : 128 * (oc + 1)],
                        rhs=qT_sb[:, dc, 512 * ic : 512 * (ic + 1)],
                        start=(dc == 0),
                        stop=(dc == DC - 1),
                    )
                # split ACT/DVE so neither queue stacks up before the exps
                if oc % 2 == 0:
                    nc.vector.tensor_copy(out=hqT_sb[:, oc, :], in_=hq_ps)
                else:
                    nc.scalar.copy(out=hqT_sb[:, oc, :], in_=hq_ps)
            return hqT_sb

        def emit_score(bst, hqT_sb, b, ic, filler=None):
            """scores, exp, denominators, reciprocal for one unit."""
            st = {"b": b, "ic": ic, "bst": bst}
            den_ps = denp.tile([32, D], F32, tag="den")
            e_tiles = []
            # two passes: all scores+exp first, then all den matmuls, so the
            # in-order PE queue never parks on a den waiting for its exp; a
            # pool chunk of the next batch between them paces PE to ACT.
            for p2 in range(4):
                sc_ps = mmp.tile([128, D], F32, tag="mm")
                nc.tensor.matmul(
                    sc_ps,
                    lhsT=bst["hkbd"][:, p2, :],
                    rhs=hqT_sb[:, p2, :],
                    start=True,
                    stop=True,
                )
                e_sb = ep.tile([128, D], BF16, tag="e")
                nc.scalar.activation(out=e_sb, in_=sc_ps, func=EXPF)
                e_tiles.append(e_sb)
            if filler:
                filler.pop(0)()
            for p2 in range(4):
                nc.tensor.matmul(
                    den_ps,
                    lhsT=ones2_sb[:, p2, :],
                    rhs=e_tiles[p2],
                    start=(p2 == 0),
                    stop=(p2 == 3),
                )
            # 1/den = exp(-ln(den)) on ACT: keeps the DVE queue free for the
            # attn multiplies and is ~3x cheaper than DVE reciprocal.
            lnd = rdp.tile([32, D], F32, tag="lnd")
            nc.scalar.activation(out=lnd, in_=den_ps, func=mybir.ActivationFunctionType.Ln)
            rden = rdp.tile([32, D], F32R, tag="rden")
            nc.scalar.activation(out=rden, in_=lnd, func=EXPF, scale=-1.0)
            st["e"] = e_tiles
            st["rden"] = rden
            return st

        def emit_B(st, filler, split_store=False):
            """normalize, V-matmul, Wo projection, store for unit st.

            The out-projection accumulates per head-chunk (start/stop over
            hc) so its matmuls interleave with the bc/mul/uv chain: the PE
            always has out-work while DVE runs the attn multiplies.  The
            DVE does ONLY the 4 multiplies; all copies go to ACT so the
            mul chain is never delayed."""
            b, ic, bst = st["b"], st["ic"], st["bst"]
            vecT_sb = vtp.tile([128, 4, D], BF16, tag="vecT")

            def bc_mul(p2):
                bc_ps = mmp.tile([128, D], F32, tag="mm")
                nc.tensor.matmul(
                    bc_ps,
                    lhsT=expand2_sb[:, p2, :],
                    rhs=st["rden"],
                    start=True,
                    stop=True,
                )
                attn_sb = attnp.tile([128, D], BF16, tag="attn")
                nc.vector.tensor_mul(attn_sb, st["e"][p2], bc_ps)
                return attn_sb

            def uv(p2, attn_sb):
                uv_ps = mmp.tile([128, D], F32, tag="mm")
                nc.tensor.matmul(
                    uv_ps,
                    lhsT=bst["hvbd"][:, p2, :],
                    rhs=attn_sb,
                    start=True,
                    stop=True,
                )
                nc.scalar.copy(out=vecT_sb[:, p2, :], in_=uv_ps)

            o_ps = {}

            def out_hc(hc):
                for it in range(IT):
                    if hc == 0:
                        o_ps[it] = mmp.tile([128, D], F32, tag="mm", name=f"ops{it}")
                    nc.tensor.matmul(
                        o_ps[it],
                        lhsT=vecT_sb[:, hc, 128 * it : 128 * (it + 1)],
                        rhs=wo_sb[:, hc, :],
                        start=(hc == 0),
                        stop=(hc == 3),
                    )

            a0 = bc_mul(0)
            a1 = bc_mul(1)
            if filler:
                filler.pop(0)()
            uv(0, a0)
            a2 = bc_mul(2)
            uv(1, a1)
            a3 = bc_mul(3)
            out_hc(0)
            uv(2, a2)
            out_hc(1)
            uv(3, a3)
            out_hc(2)
            if filler:
                filler.pop(0)()
            out_hc(3)
            out_sb = outp.tile([128, IT, D], F32, tag="o")
            for it in range(IT):
                if it % 2 == 0:
                    nc.vector.tensor_copy(out=out_sb[:, it, :], in_=o_ps[it])
                else:
                    nc.scalar.copy(out=out_sb[:, it, :], in_=o_ps[it])
                if split_store:
                    nc.scalar.dma_start(
                        out=out_v[b, ic][:, it, :], in_=out_sb[:, it, :]
                    )
            if not split_store:
                # store on the second HWDGE ring (ACT) so loads/stores overlap
                nc.scalar.dma_start(out=out_v[b, ic], in_=out_sb)

        # ------------------- software pipeline -------------------
        # batch 0: kv quarter-tiles on the sync ring; qT0/wq ride the idle
        # scalar ring in parallel; hq(0,0) halves woven between kv arrivals
        steps0, bst0 = make_prep(0, halves=True)
        qT0 = qtp.tile([128, DC, QLEN], BF16, tag="qT", name="qT0")
        nc.scalar.dma_start(out=qT0, in_=q_v[0])
        nc.scalar.dma_start(out=wq_sb, in_=wq[:, :, :])
        for _ in range(4):
            steps0.pop(0)()
        hqT_00 = emit_hq(bst0, qT0, 0, ocs=[0, 1])
        for _ in range(4):
            steps0.pop(0)()
        emit_hq(bst0, qT0, 0, ocs=[2, 3], hqT_sb=hqT_00)
        for _ in range(4):
            steps0.pop(0)()
        nc.sync.dma_start(out=wk_sb, in_=wk[:, :, :])
        nc.sync.dma_start(out=wv_sb, in_=wv[:, :, :])
        for _ in range(4):
            steps0.pop(0)()
        nc.scalar.dma_start(out=ones2_sb, in_=ones2[:, :, :])
        nc.scalar.dma_start(out=expand2_sb, in_=expand2[:, :, :])
        tail_a(bst0)
        hqT_01 = emit_hq(bst0, qT0, 1)
        nc.sync.dma_start(out=wo_sb, in_=wo[:, :, :])
        tail_b(bst0)

        batch_state = {0: bst0}
        hq_pre = {(0, 0): hqT_00, (0, 1): hqT_01}
        qT_cur = qT0
        for b in range(B):
            bst = batch_state[b]
            if b > 0:
                hq_pre[(b, 0)] = emit_hq(bst, qT_cur, 0)
                tail_a(bst)
                if b < B - 1:
                    # last batch defers hq(b,1) to fill the score phase
                    # (it has no next-batch pool chunks to weave)
                    hq_pre[(b, 1)] = emit_hq(bst, qT_cur, 1)
                tail_b(bst)
            if b + 1 < B:
                next_steps, next_bst = make_prep(b + 1)
                qT_next = qtp.tile([128, DC, QLEN], BF16, tag="qT")
                nc.sync.dma_start(out=qT_next, in_=q_v[b + 1])
            else:
                next_steps, next_bst, qT_next = [], None, None
            st0 = emit_score(bst, hq_pre.pop((b, 0)), b, 0, next_steps)
            if (b, 1) not in hq_pre:
                hq_pre[(b, 1)] = emit_hq(bst, qT_cur, 1)
            st1 = emit_score(bst, hq_pre.pop((b, 1)), b, 1, next_steps)
            # two more chunks here cover the den->rden ACT latency before
            # B(b,0)'s first broadcast matmul needs rden.
            for _ in range(min(2, len(next_steps))):
                next_steps.pop(0)()
            emit_B(st0, next_steps)
            emit_B(st1, next_steps, split_store=(b == B - 1))
            if next_bst is not None:
                batch_state[b + 1] = next_bst
                qT_cur = qT_next
    return nc


_NC = None


def _get_nc() -> bass.Bass:
    global _NC
    if _NC is None:
        _NC = _build_nc()
    return _NC


def _consts() -> dict:
    # pooling band: slice [:, 60-4g : 124-4g] has 1/64 at column (4g + p//32)
    poolD = np.zeros((128, 124), np.float32)
    for p in range(128):
        poolD[p, 60 + p // 32] = 1.0 / 64.0
    ident = np.eye(MEM, dtype=np.float32)
    # den matmul lhsT (per pair p2): row 2*p2 sums even-head exp rows
    # (partitions 0-63), row 2*p2+1 sums odd (64-127); rows 8-31 duplicate
    # row 0's pattern so every PSUM row stays finite positive (the
    # approx-reciprocal runs on the whole tile and inf/0 are undefined).
    ones2 = np.zeros((128, 4, 32), np.float32)
    for p2 in range(4):
        ones2[0:64, p2, 2 * p2] = 1.0
        ones2[64:128, p2, 2 * p2 + 1] = 1.0
        if p2 == 0:
            ones2[0:64, p2, 8:32] = 1.0
    # broadcast matmul lhsT, one [32, 128] slice per head pair
    expand2 = np.zeros((32, 4, 128), np.float32)
    for p2 in range(4):
        expand2[2 * p2 + 0, p2, 0:64] = 1.0
        expand2[2 * p2 + 1, p2, 64:128] = 1.0
    return {
        "poolD": poolD.astype(NPBF16),
        "ident": ident.astype(NPBF16),
        "ones2": ones2.astype(NPBF16),
        "expand2": expand2,
    }


def _w_layout(wT: np.ndarray, scale: float = 1.0) -> np.ndarray:
    """[D, D] (already transposed) -> [128, DC, D] bf16, contiguous."""
    w = (wT * scale).astype(NPBF16) if scale != 1.0 else wT.astype(NPBF16)
    return np.ascontiguousarray(w.reshape(DC, 128, D).transpose(1, 0, 2))


def run(inputs: dict, trace: bool = False):
    """Run on 8 cores; returns (full_output, BassKernelResults)."""
    query = np.asarray(inputs["query"], np.float32)
    queryT = np.ascontiguousarray(query.transpose(0, 2, 1)).astype(NPBF16)
    keyvalue = np.asarray(inputs["keyvalue"], np.float32).astype(NPBF16)
    w = {
        "wq": _w_layout(np.asarray(inputs["Wq"], np.float32).T),
        # fold the 1/sqrt(hd)=1/8 score scale into Wk (exact in bf16)
        "wk": _w_layout(np.asarray(inputs["Wk"], np.float32).T, 0.125),
        "wv": _w_layout(np.asarray(inputs["Wv"], np.float32).T),
        "wo": _w_layout(np.asarray(inputs["Wo"], np.float32).T),
    }
    consts = _consts()
    nb = query.shape[0]
    per = nb // NCORES
    assert per == B, f"expected {NCORES * B} batches, got {nb}"

    in_maps = []
    for k in range(NCORES):
        m = {
            "queryT": np.ascontiguousarray(queryT[k * per : (k + 1) * per]),
            "keyvalue": np.ascontiguousarray(keyvalue[k * per : (k + 1) * per]),
        }
        m.update(w)
        m.update(consts)
        in_maps.append(m)

    res = run_bass_kernel_spmd(
        _get_nc(), in_maps, core_ids=list(range(NCORES)), trace=trace
    )
    outs = [r["out"] for r in res.results]
    return np.concatenate(outs, axis=0), res


def kernel(**inputs) -> np.ndarray:
    out, _ = run(inputs, trace=False)
    return out


# revision 28
# speedup vs baseline: 1.0387x; 1.0171x over previous
"""Trainium2 Bass kernel for MultiHeadMemAttn (mean-pooled-memory attention).

Full computation (per batch b):
    mem  = mean_pool(keyvalue, window=64, stride=64)          # [64, 512]
    hq   = query @ Wq.T ; hk = mem @ Wk.T ; hv = mem @ Wv.T   # heads=8, hd=64
    attn = softmax(hq @ hk.T / 8, over mem axis)
    out  = (attn @ hv) @ Wo.T

Sharding: data-parallel over batch across 8 cores (4 batches each),
weights replicated.  No collectives.

v2 strategy (per core):
  - ALL inputs are cast to bf16 on the host before staging: HBM traffic
    drops from 52MB to 30MB per core (the kernel is memory-regime).
    Matmul speed is identical to f32r (1 cycle/row), precision ~1e-3.
  - kv is loaded with 2 consecutive s-rows per partition so descriptors
    stay 2KB in bf16; pooling is a PE matmul against a shifted band
    matrix, accumulated over 32 half-chunks in PSUM.
  - weights are host-pre-arranged to [128, dc, 512] so each loads with
    one fully-contiguous DMA.
  - scores computed transposed (scoresT[j, i] = hkbd_p2.T @ hqT_p2), head
    pairs packed into [128, 512] tiles; softmax without max-subtraction
    (|scores| <= ~6): E = exp(s/8) on ACT (1/8 folded into host Wk),
    denominators via a ones-matmul -> [32,512], reciprocal_approx_fast on
    DVE (5x faster than reciprocal), partition-broadcast via a tiny
    expand-matmul, normalize on DVE.
  - hkbd / hvbd block-diagonal tiles are persistent ping-pong buffers;
    the zero off-diagonal blocks are written once at startup.
  - software pipeline: pool chunks of batch b+1 are woven into the
    B-phases of batch b; hq is emitted between the tail sub-steps so the
    PE never waits on ACT copies; scores of both i-chunks run before
    either B-phase so the reciprocal latency is hidden.
"""

import os
from contextlib import ExitStack

import numpy as np
import ml_dtypes

import concourse.bass as bass
import concourse.mybir as mybir
import concourse.tile as tile
from concourse.bass_utils import run_bass_kernel_spmd

F32 = mybir.dt.float32
F32R = mybir.dt.float32r
BF16 = mybir.dt.bfloat16
NPBF16 = ml_dtypes.bfloat16

NCORES = 8
B = 4          # batches per core
QLEN = 1024
S = 4096       # kv sequence length
D = 512        # hidden
H = 8          # heads
HD = 64        # head dim
MEM = 64       # mem_len (pooled length)
DC = D // 128  # 4 chunks of the hidden dim
ICN = 2        # i-chunks of 512 per batch
IT = 4         # 128-row tiles per i-chunk
KT = 8         # kv DMA tiles per batch (each 512 s-rows: 2 groups x 128p x 2rows)

EXPF = mybir.ActivationFunctionType.Exp

# ---------------------------------------------------------------------------
# Workaround: this walrus build only encodes ONE sem-wait per instruction
# ("Too many sync wait commands" in CoreV3GenImpl setupSyncWait), while
# Tile's sem-assignment freely attaches several.  Post-process the
# serialized BIR: move surplus waits onto injected same-engine NoOps placed
# immediately before the instruction (engine streams are in-order, so the
# NoOp chain stalls the engine exactly like multi-wait would).
import json as _json

_orig_to_json_bytes = bass.Bass.to_json_bytes


def _split_multi_waits(self, *args, **kwargs):
    raw = _orig_to_json_bytes(self, *args, **kwargs)
    d = _json.loads(raw)
    changed = False

    def fix_block(o):
        nonlocal changed
        if isinstance(o, dict):
            insts = o.get("instructions")
            if isinstance(insts, list):
                new = []
                for inst in insts:
                    si = inst.get("sync_info") if isinstance(inst, dict) else None
                    waits = (si or {}).get("on_wait") or []
                    if len(waits) > 1:
                        changed = True
                        for i, w in enumerate(waits[:-1]):
                            new.append(
                                {
                                    "name": f"{inst['name']}-sw{i}",
                                    "opcode": "NoOp",
                                    "engine": inst["engine"],
                                    "ins": [],
                                    "outs": [],
                                    "debug": inst.get("debug", 0),
                                    "sync_info": {
                                        "on_wait": [w],
                                        "on_update": [],
                                    },
                                }
                            )
                        si["on_wait"] = [waits[-1]]
                    new.append(inst)
                o["instructions"] = new
            for v in o.values():
                fix_block(v)
        elif isinstance(o, list):
            for v in o:
                fix_block(v)

    fix_block(d)
    if not changed:
        return raw
    return _json.dumps(d).encode()


bass.Bass.to_json_bytes = _split_multi_waits
# ---------------------------------------------------------------------------


def _build_nc() -> bass.Bass:
    nc = bass.Bass()
    q = nc.dram_tensor("queryT", [B, D, QLEN], BF16, kind="ExternalInput")
    kv = nc.dram_tensor("keyvalue", [B, S, D], BF16, kind="ExternalInput")
    wq = nc.dram_tensor("wq", [128, DC, D], BF16, kind="ExternalInput")
    wk = nc.dram_tensor("wk", [128, DC, D], BF16, kind="ExternalInput")
    wv = nc.dram_tensor("wv", [128, DC, D], BF16, kind="ExternalInput")
    wo = nc.dram_tensor("wo", [128, DC, D], BF16, kind="ExternalInput")
    poolD = nc.dram_tensor("poolD", [128, 124], BF16, kind="ExternalInput")
    ident = nc.dram_tensor("ident", [MEM, MEM], BF16, kind="ExternalInput")
    ones2 = nc.dram_tensor("ones2", [128, 4, 64], BF16, kind="ExternalInput")
    expand2 = nc.dram_tensor("expand2", [64, 4, 128], F32R, kind="ExternalInput")
    out = nc.dram_tensor("out", [B, QLEN, D], F32, kind="ExternalOutput")

    # DRAM views for partition-major DMA (2KB descriptors everywhere)
    q_v = q.rearrange("b (dc p) i -> b p dc i", p=128)
    kv_v = kv.rearrange("b (t c p two) d -> b t p c two d", t=KT, c=2, p=128, two=2)
    out_v = out.rearrange("b (ic it p) d -> b ic p it d", ic=ICN, it=IT, p=128)

    with tile.TileContext(nc) as tc, ExitStack() as ctx:
        # SBUF pools
        singles = ctx.enter_context(tc.tile_pool(name="singles", bufs=1))
        kvp = ctx.enter_context(tc.tile_pool(name="kvp", bufs=8))
        qtp = ctx.enter_context(tc.tile_pool(name="qtp", bufs=2))
        hqp = ctx.enter_context(tc.tile_pool(name="hqp", bufs=2))
        memp = ctx.enter_context(tc.tile_pool(name="memp", bufs=2))
        ep = ctx.enter_context(tc.tile_pool(name="ep", bufs=9))
        attnp = ctx.enter_context(tc.tile_pool(name="attnp", bufs=4))
        rdp = ctx.enter_context(tc.tile_pool(name="rdp", bufs=3))
        vtp = ctx.enter_context(tc.tile_pool(name="vtp", bufs=3))
        outp = ctx.enter_context(tc.tile_pool(name="outp", bufs=2))
        # PSUM pools (8 banks total: 1 + 1 + 6)
        accp = ctx.enter_context(tc.tile_pool(name="accp", bufs=1, space="PSUM"))
        denp = ctx.enter_context(tc.tile_pool(name="denp", bufs=1, space="PSUM"))
        mmp = ctx.enter_context(tc.tile_pool(name="mmp", bufs=6, space="PSUM"))

        # --- one-time loads; small consts go on the scalar (ACT) HWDGE ring
        # so the first kv tile starts on the sync ring immediately ---
        poolD_sb = singles.tile([128, 124], BF16)
        nc.scalar.dma_start(out=poolD_sb, in_=poolD[:, :])
        ident_sb = singles.tile([MEM, MEM], BF16)
        nc.scalar.dma_start(out=ident_sb, in_=ident[:, :])

        # persistent block-diagonal tiles (ping-pong); zero the whole tiles
        # once, per-batch writes touch only the diagonal blocks.
        hkbd_t = [singles.tile([128, 4, 128], BF16, name=f"hkbd{i}") for i in range(2)]
        hvbd_t = [singles.tile([128, 4, 128], BF16, name=f"hvbd{i}") for i in range(2)]
        for t in hkbd_t + hvbd_t:
            nc.gpsimd.memset(t[:, :, :], 0.0)

        wq_sb = singles.tile([128, DC, D], BF16)
        wk_sb = singles.tile([128, DC, D], BF16)
        wv_sb = singles.tile([128, DC, D], BF16)
        wo_sb = singles.tile([128, DC, D], BF16)
        ones2_sb = singles.tile([128, 4, 64], BF16)
        expand2_sb = singles.tile([64, 4, 128], F32R)

        def make_prep(b, halves=False):
            """Returns (chunk_steps, state).  Each chunk step loads one kv
            tile (512 s-rows as [128p, 2c, 2rows, 512d]) and runs its 4 pool
            matmuls; steps are interleaved into other units' emission.
            halves=True (batch 0) uses 16 quarter-MB steps so the first
            pool matmul starts as early as possible."""
            st = {"b": b, "bd": b % 2}
            pacc = accp.tile([MEM, D], F32, tag="acc")
            st["pacc"] = pacc

            def chunk_step(t):
                def f():
                    kvt = kvp.tile([128, 2, 2, D], BF16, tag="kv")
                    nc.sync.dma_start(out=kvt, in_=kv_v[b, t])
                    for c in range(2):
                        g = 2 * t + c
                        for two in range(2):
                            nc.tensor.matmul(
                                pacc,
                                lhsT=poolD_sb[:, 60 - 4 * g : 124 - 4 * g],
                                rhs=kvt[:, c, two, :],
                                start=(g == 0 and two == 0),
                                stop=(g == 15 and two == 1),
                            )
                return f

            def half_step(t, c):
                def f():
                    kvh = kvp.tile([128, 2, D], BF16, tag="kvh")
                    nc.sync.dma_start(out=kvh, in_=kv_v[b, t][:, c, :, :])
                    g = 2 * t + c
                    for two in range(2):
                        nc.tensor.matmul(
                            pacc,
                            lhsT=poolD_sb[:, 60 - 4 * g : 124 - 4 * g],
                            rhs=kvh[:, two, :],
                            start=(g == 0 and two == 0),
                            stop=(g == 15 and two == 1),
                        )
                return f

            if halves:
                return [half_step(t, c) for t in range(KT) for c in range(2)], st
            return [chunk_step(t) for t in range(KT)], st

        def tail_a(bst):
            """mem copy + PE transposes + memT copy."""
            mem_sb = memp.tile([MEM, D], BF16, tag="mem")
            nc.scalar.copy(out=mem_sb, in_=bst["pacc"])
            trt = mmp.tile([128, 4, MEM], BF16, tag="mm")
            for c in range(4):
                nc.tensor.transpose(
                    trt[:, c, :],
                    mem_sb[:, 128 * c : 128 * (c + 1)],
                    ident_sb[:, :],
                )
            memT_sb = memp.tile([128, 4, MEM], BF16, tag="memT")
            nc.scalar.copy(out=memT_sb, in_=trt)
            bst["memT"] = memT_sb

        def tail_b(bst):
            """hk -> hkbd diag, hv -> hvbd diag (ping-pong tiles)."""
            memT_sb = bst["memT"]
            hkbd_sb = hkbd_t[bst["bd"]]
            hvbd_sb = hvbd_t[bst["bd"]]
            for oc in range(4):
                hk_ps = mmp.tile([128, MEM], F32, tag="mm")
                for dc in range(DC):
                    nc.tensor.matmul(
                        hk_ps,
                        lhsT=wk_sb[:, dc, 128 * oc : 128 * (oc + 1)],
                        rhs=memT_sb[:, dc, :],
                        start=(dc == 0),
                        stop=(dc == DC - 1),
                    )
                # 1/8 score scale is folded into host wk
                nc.scalar.copy(out=hkbd_sb[0:64, oc, 0:64], in_=hk_ps[0:64, :])
                nc.scalar.copy(out=hkbd_sb[64:128, oc, 64:128], in_=hk_ps[64:128, :])
            hv_ps = mmp.tile([MEM, D], F32, tag="mm")
            for dc in range(DC):
                nc.tensor.matmul(
                    hv_ps,
                    lhsT=memT_sb[:, dc, :],
                    rhs=wv_sb[:, dc, :],
                    start=(dc == 0),
                    stop=(dc == DC - 1),
                )
            hv_sb = memp.tile([MEM, D], BF16, tag="hv")
            nc.scalar.copy(out=hv_sb, in_=hv_ps)
            ev = hv_sb.rearrange("m (p2 two dv) -> m p2 two dv", p2=4, two=2)
            nc.scalar.copy(out=hvbd_sb[0:64, :, 0:64], in_=ev[:, :, 0, :])
            nc.sync.dma_start(out=hvbd_sb[64:128, :, 64:128], in_=ev[:, :, 1, :])
            bst["hkbd"] = hkbd_sb
            bst["hvbd"] = hvbd_sb

        def emit_hq(bst, qT_sb, ic, ocs=None, hqT_sb=None):
            """hqT[:, oc, :] = Wq-chunks.T @ qT for one 512-wide i-chunk."""
            if hqT_sb is None:
                hqT_sb = hqp.tile([128, DC, D], BF16, tag="hqT", name="hqT_t")
            for oc in ocs if ocs is not None else range(DC):
                hq_ps = mmp.tile([128, D], F32, tag="mm")
                for dc in range(DC):
                    nc.tensor.matmul(
                        hq_ps,
                        lhsT=wq_sb[:, dc, 128 * oc : 128 * (oc + 1)],
                        rhs=qT_sb[:, dc, 512 * ic : 512 * (ic + 1)],
                        start=(dc == 0),
                        stop=(dc == DC - 1),
                    )
                # split ACT/DVE so neither queue stacks up before the exps
                if oc % 2 == 0:
                    nc.vector.tensor_copy(out=hqT_sb[:, oc, :], in_=hq_ps)
                else:
                    nc.scalar.copy(out=hqT_sb[:, oc, :], in_=hq_ps)
            return hqT_sb

        def emit_score(bst, hqT_sb, b, ic, filler=None):
            """scores, exp, denominators, reciprocal for one unit."""
            st = {"b": b, "ic": ic, "bst": bst}
            den_ps = denp.tile([64, D], F32, tag="den")
            e_tiles = []
            # two passes: all scores+exp first, then all den matmuls, so the
            # in-order PE queue never parks on a den waiting for its exp; a
            # pool chunk of the next batch between them paces PE to ACT.
            for p2 in range(4):
                sc_ps = mmp.tile([128, D], F32, tag="mm")
                nc.tensor.matmul(
                    sc_ps,
                    lhsT=bst["hkbd"][:, p2, :],
                    rhs=hqT_sb[:, p2, :],
                    start=True,
                    stop=True,
                )
                e_sb = ep.tile([128, D], BF16, tag="e")
                nc.scalar.activation(out=e_sb, in_=sc_ps, func=EXPF)
                e_tiles.append(e_sb)
            if filler:
                filler.pop(0)()
            for p2 in range(4):
                nc.tensor.matmul(
                    den_ps,
                    lhsT=ones2_sb[:, p2, :],
                    rhs=e_tiles[p2],
                    start=(p2 == 0),
                    stop=(p2 == 3),
                )
            # 1/den = exp(-ln(den)) on ACT: keeps the DVE queue free for the
            # attn multiplies and is ~3x cheaper than DVE reciprocal.
            lnd = rdp.tile([64, D], F32, tag="lnd")
            nc.scalar.activation(out=lnd, in_=den_ps, func=mybir.ActivationFunctionType.Ln)
            rden = rdp.tile([64, D], F32R, tag="rden")
            nc.scalar.activation(out=rden, in_=lnd, func=EXPF, scale=-1.0)
            st["e"] = e_tiles
            st["rden"] = rden
            return st

        def emit_B(st, filler, split_store=False):
            """normalize, V-matmul, Wo projection, store for unit st.

            The out-projection accumulates per head-chunk (start/stop over
            hc) so its matmuls interleave with the bc/mul/uv chain: the PE
            always has out-work while DVE runs the attn multiplies.  The
            DVE does ONLY the 4 multiplies; all copies go to ACT so the
            mul chain is never delayed."""
            b, ic, bst = st["b"], st["ic"], st["bst"]
            vecT_sb = vtp.tile([128, 4, D], BF16, tag="vecT")

            def bc_mul(p2):
                bc_ps = mmp.tile([128, D], F32, tag="mm")
                nc.tensor.matmul(
                    bc_ps,
                    lhsT=expand2_sb[:, p2, :],
                    rhs=st["rden"],
                    start=True,
                    stop=True,
                )
                attn_sb = attnp.tile([128, D], BF16, tag="attn")
                nc.vector.tensor_mul(attn_sb, st["e"][p2], bc_ps)
                return attn_sb

            def uv(p2, attn_sb):
                uv_ps = mmp.tile([128, D], F32, tag="mm")
                nc.tensor.matmul(
                    uv_ps,
                    lhsT=bst["hvbd"][:, p2, :],
                    rhs=attn_sb,
                    start=True,
                    stop=True,
                )
                nc.scalar.copy(out=vecT_sb[:, p2, :], in_=uv_ps)

            o_ps = {}

            def out_hc(hc):
                for it in range(IT):
                    if hc == 0:
                        o_ps[it] = mmp.tile([128, D], F32, tag="mm", name=f"ops{it}")
                    nc.tensor.matmul(
                        o_ps[it],
                        lhsT=vecT_sb[:, hc, 128 * it : 128 * (it + 1)],
                        rhs=wo_sb[:, hc, :],
                        start=(hc == 0),
                        stop=(hc == 3),
                    )

            a0 = bc_mul(0)
            a1 = bc_mul(1)
            if filler:
                filler.pop(0)()
            uv(0, a0)
            a2 = bc_mul(2)
            uv(1, a1)
            a3 = bc_mul(3)
            out_hc(0)
            uv(2, a2)
            out_hc(1)
            uv(3, a3)
            out_hc(2)
            if filler:
                filler.pop(0)()
            out_hc(3)
            out_sb = outp.tile([128, IT, D], F32, tag="o")
            for it in range(IT):
                if it % 2 == 0:
                    nc.vector.tensor_copy(out=out_sb[:, it, :], in_=o_ps[it])
                else:
                    nc.scalar.copy(out=out_sb[:, it, :], in_=o_ps[it])
                if split_store:
                    nc.scalar.dma_start(
                        out=out_v[b, ic][:, it, :], in_=out_sb[:, it, :]
                    )
            if not split_store:
                # store on the second HWDGE ring (ACT) so loads/stores overlap
                nc.scalar.dma_start(out=out_v[b, ic], in_=out_sb)

        # ------------------- software pipeline -------------------
        # batch 0: kv quarter-tiles on the sync ring; qT0/wq ride the idle
        # scalar ring in parallel; hq(0,0) halves woven between kv arrivals
        steps0, bst0 = make_prep(0, halves=True)
        qT0 = qtp.tile([128, DC, QLEN], BF16, tag="qT", name="qT0")
        nc.scalar.dma_start(out=qT0, in_=q_v[0])
        nc.scalar.dma_start(out=wq_sb, in_=wq[:, :, :])
        for _ in range(4):
            steps0.pop(0)()
        hqT_00 = emit_hq(bst0, qT0, 0, ocs=[0, 1])
        for _ in range(4):
            steps0.pop(0)()
        emit_hq(bst0, qT0, 0, ocs=[2, 3], hqT_sb=hqT_00)
        for _ in range(4):
            steps0.pop(0)()
        nc.sync.dma_start(out=wk_sb, in_=wk[:, :, :])
        nc.sync.dma_start(out=wv_sb, in_=wv[:, :, :])
        for _ in range(4):
            steps0.pop(0)()
        nc.scalar.dma_start(out=ones2_sb, in_=ones2[:, :, :])
        nc.scalar.dma_start(out=expand2_sb, in_=expand2[:, :, :])
        tail_a(bst0)
        hqT_01 = emit_hq(bst0, qT0, 1)
        nc.sync.dma_start(out=wo_sb, in_=wo[:, :, :])
        tail_b(bst0)

        batch_state = {0: bst0}
        hq_pre = {(0, 0): hqT_00, (0, 1): hqT_01}
        qT_cur = qT0
        for b in range(B):
            bst = batch_state[b]
            if b > 0:
                hq_pre[(b, 0)] = emit_hq(bst, qT_cur, 0)
                tail_a(bst)
                if b < B - 1:
                    # last batch defers hq(b,1) to fill the score phase
                    # (it has no next-batch pool chunks to weave)
                    hq_pre[(b, 1)] = emit_hq(bst, qT_cur, 1)
                tail_b(bst)
            if b + 1 < B:
                next_steps, next_bst = make_prep(b + 1)
                qT_next = qtp.tile([128, DC, QLEN], BF16, tag="qT")
                nc.sync.dma_start(out=qT_next, in_=q_v[b + 1])
            else:
                next_steps, next_bst, qT_next = [], None, None
            st0 = emit_score(bst, hq_pre.pop((b, 0)), b, 0, next_steps)
            if (b, 1) not in hq_pre:
                hq_pre[(b, 1)] = emit_hq(bst, qT_cur, 1)
            st1 = emit_score(bst, hq_pre.pop((b, 1)), b, 1, next_steps)
            # two more chunks here cover the den->rden ACT latency before
            # B(b,0)'s first broadcast matmul needs rden.
            for _ in range(min(2, len(next_steps))):
                next_steps.pop(0)()
            emit_B(st0, next_steps)
            emit_B(st1, next_steps, split_store=(b == B - 1))
            if next_bst is not None:
                batch_state[b + 1] = next_bst
                qT_cur = qT_next
    return nc


_NC = None


def _get_nc() -> bass.Bass:
    global _NC
    if _NC is None:
        _NC = _build_nc()
    return _NC


def _consts() -> dict:
    # pooling band: slice [:, 60-4g : 124-4g] has 1/64 at column (4g + p//32)
    poolD = np.zeros((128, 124), np.float32)
    for p in range(128):
        poolD[p, 60 + p // 32] = 1.0 / 64.0
    ident = np.eye(MEM, dtype=np.float32)
    # den matmul lhsT (per pair p2): row 2*p2 sums even-head exp rows
    # (partitions 0-63), row 2*p2+1 sums odd (64-127); rows 8-31 duplicate
    # row 0's pattern so every PSUM row stays finite positive (the
    # approx-reciprocal runs on the whole tile and inf/0 are undefined).
    ones2 = np.zeros((128, 4, 64), np.float32)
    for p2 in range(4):
        ones2[0:64, p2, 2 * p2] = 1.0
        ones2[64:128, p2, 2 * p2 + 1] = 1.0
        if p2 == 0:
            # pad rows 8-63 of den with a positive value so Ln/Exp stay
            # finite (the padded bc rows multiply by zero in expand2)
            ones2[0:64, p2, 8:64] = 1.0
    # broadcast matmul lhsT, one [32, 128] slice per head pair
    expand2 = np.zeros((64, 4, 128), np.float32)
    for p2 in range(4):
        expand2[2 * p2 + 0, p2, 0:64] = 1.0
        expand2[2 * p2 + 1, p2, 64:128] = 1.0
    return {
        "poolD": poolD.astype(NPBF16),
        "ident": ident.astype(NPBF16),
        "ones2": ones2.astype(NPBF16),
        "expand2": expand2,
    }


def _w_layout(wT: np.ndarray, scale: float = 1.0) -> np.ndarray:
    """[D, D] (already transposed) -> [128, DC, D] bf16, contiguous."""
    w = (wT * scale).astype(NPBF16) if scale != 1.0 else wT.astype(NPBF16)
    return np.ascontiguousarray(w.reshape(DC, 128, D).transpose(1, 0, 2))


def run(inputs: dict, trace: bool = False):
    """Run on 8 cores; returns (full_output, BassKernelResults)."""
    query = np.asarray(inputs["query"], np.float32)
    queryT = np.ascontiguousarray(query.transpose(0, 2, 1)).astype(NPBF16)
    keyvalue = np.asarray(inputs["keyvalue"], np.float32).astype(NPBF16)
    w = {
        "wq": _w_layout(np.asarray(inputs["Wq"], np.float32).T),
        # fold the 1/sqrt(hd)=1/8 score scale into Wk (exact in bf16)
        "wk": _w_layout(np.asarray(inputs["Wk"], np.float32).T, 0.125),
        "wv": _w_layout(np.asarray(inputs["Wv"], np.float32).T),
        "wo": _w_layout(np.asarray(inputs["Wo"], np.float32).T),
    }
    consts = _consts()
    nb = query.shape[0]
    per = nb // NCORES
    assert per == B, f"expected {NCORES * B} batches, got {nb}"

    in_maps = []
    for k in range(NCORES):
        m = {
            "queryT": np.ascontiguousarray(queryT[k * per : (k + 1) * per]),
            "keyvalue": np.ascontiguousarray(keyvalue[k * per : (k + 1) * per]),
        }
        m.update(w)
        m.update(consts)
        in_maps.append(m)

    res = run_bass_kernel_spmd(
        _get_nc(), in_maps, core_ids=list(range(NCORES)), trace=trace
    )
    outs = [r["out"] for r in res.results]
    return np.concatenate(outs, axis=0), res


def kernel(**inputs) -> np.ndarray:
    out, _ = run(inputs, trace=False)
    return out
